# revision 18
# baseline (speedup 1.0000x reference)
"""PointUpsampleAttn (3-NN gather attention) Trainium2 kernel.

Full-input contract: kernel(q, k, v) -> [B, C, N] float32.
  q [4, 16384, 3], k [4, 4096, 3], v [4, 4096, 256]

Sharding: B*N = 65536 queries split across 8 cores (8192 each); core c
handles batch c//2, query half c%2. k/v replicated per-batch (each core
only needs its own batch's k/v). No cross-core reduction.

Per-core kernel, per 128-query tile:
  1. PE matmul (K=11, fp16 hi/lo split of q / 2k / -|k|^2) -> m = 2qk-kk
     in PSUM [128, 4096], fp32-class accuracy at full bf16 PE rate.
  2. ACT copies PSUM -> SBUF.
  3. DVE max8 + max_index -> top-3 m values + s-indices.
  4. weights w = normalize(1/(qq+eps-m_top3)).
  5. 3x indirect DMA gather of v rows; weighted sum; PE transpose to
     [C, n] layout; DMA to output.
"""

import numpy as np

B, N, S, C = 4, 16384, 4096, 256
NCORES = 8
NSH = (B * N) // NCORES   # 8192 queries per core
PT = 128                  # queries per tile (partition dim)
NT = NSH // PT            # 64 tiles
KROWS = 21                # contraction rows of the split matmul

_CACHE = {}


def _build_bass():
    import concourse.bacc as bacc
    import concourse.mybir as mybir
    import concourse.tile as tile
    from concourse import bass
    from concourse.masks import make_identity

    f32 = mybir.dt.float32
    f16 = mybir.dt.float16
    u32 = mybir.dt.uint32

    nc = bacc.Bacc("TRN2", target_bir_lowering=False, debug=False)

    a_d = nc.dram_tensor("a", [KROWS, NSH], f16, kind="ExternalInput").ap()
    k_d = nc.dram_tensor("kaug", [KROWS, S], f16, kind="ExternalInput").ap()
    qq_d = nc.dram_tensor("qq", [PT, NT], f32, kind="ExternalInput").ap()
    v_d = nc.dram_tensor("v", [S, C], f32, kind="ExternalInput").ap()
    out_d = nc.dram_tensor("out", [C, NSH], f32, kind="ExternalOutput").ap()

    with tile.TileContext(nc) as tc:
        with (
            tc.tile_pool(name="const", bufs=1) as cpool,
            tc.tile_pool(name="m", bufs=2) as mpool,
            tc.tile_pool(name="sel", bufs=3) as spool,
            tc.tile_pool(name="g", bufs=3) as gpool,
            tc.tile_pool(name="o", bufs=3) as opool,
            tc.tile_pool(name="mm", bufs=3, space="PSUM") as psum_mm,
            tc.tile_pool(name="tp", bufs=2, space="PSUM") as psum_tp,
        ):
            a_sb = cpool.tile([KROWS, NSH], f16)
            nc.sync.dma_start(a_sb[:], a_d[:])
            k_sb = cpool.tile([KROWS, S], f16)
            nc.sync.dma_start(k_sb[:], k_d[:])
            qq_sb = cpool.tile([PT, NT], f32)
            nc.sync.dma_start(qq_sb[:], qq_d[:])
            ident = cpool.tile([PT, PT], f32)
            make_identity(nc, ident[:])

            G4 = 4  # tiles per weight-batch group
            for gi in range(NT // G4):
                idxs = []
                # d12[:, 3t:3t+3] = relu(qq_t - m_top3_t) = d_t + 1e-8 (clamped)
                d12 = spool.tile([PT, 3 * G4], f32, tag="d12")
                for t in range(G4):
                    i = gi * G4 + t
                    # 1. distances: m = 2 q.k - |k|^2 for 128 queries
                    m_sb = mpool.tile([PT, S], f32, tag="m")
                    lhsT = a_sb[:, i * PT:(i + 1) * PT]
                    for j in range(S // 1024):
                        ps = psum_mm.tile([PT, 1024], f32, tag="mm")
                        for jj in range(2):
                            nc.tensor.matmul(
                                ps[:, jj * 512:(jj + 1) * 512], lhsT,
                                k_sb[:, j * 1024 + jj * 512:j * 1024 + (jj + 1) * 512],
                                start=True, stop=True,
                            )
                        # 2. PSUM -> SBUF on the scalar engine
                        nc.scalar.copy(m_sb[:, j * 1024:(j + 1) * 1024], ps[:])

                    # 3. top-8 values + indices (we use the first 3)
                    top8 = spool.tile([PT, 8], f32, tag=f"top8_{t}")
                    nc.vector.max(out=top8[:], in_=m_sb[:])
                    idx8 = spool.tile([PT, 8], u32, tag=f"idx8_{t}")
                    nc.vector.max_index(out=idx8[:], in_max=top8[:], in_values=m_sb[:])
                    idxs.append(idx8)

                    nc.scalar.activation(
                        out=d12[:, 3 * t:3 * t + 3], in_=top8[:, 0:3],
                        func=mybir.ActivationFunctionType.Relu,
                        scale=-1.0, bias=qq_sb[:, i:i + 1],
                    )

                # 4. weights for the whole group: d floored at 1e-9 (guards the
                # relu-zeroed pathological case), r = 1/d, w = r / sum(r)
                nc.vector.tensor_scalar_max(d12[:], d12[:], 1e-9)
                r12 = spool.tile([PT, 3 * G4], f32, tag="r12")
                nc.vector.reciprocal(r12[:], d12[:])
                z4 = spool.tile([PT, G4], f32, tag="z4")
                nc.vector.tensor_reduce(
                    out=z4[:], in_=r12[:].rearrange("p (a b) -> p a b", b=3),
                    axis=mybir.AxisListType.X, op=mybir.AluOpType.add,
                )
                rz4 = spool.tile([PT, G4], f32, tag="rz4")
                nc.vector.reciprocal(rz4[:], z4[:])
                w12 = spool.tile([PT, 3 * G4], f32, tag="w12")
                for t in range(G4):
                    nc.vector.tensor_scalar(
                        out=w12[:, 3 * t:3 * t + 3], in0=r12[:, 3 * t:3 * t + 3],
                        scalar1=rz4[:, t:t + 1], scalar2=None,
                        op0=mybir.AluOpType.mult,
                    )

                for t in range(G4):
                    i = gi * G4 + t
                    idx8 = idxs[t]
                    # 5. gather v rows (one indirect DMA per neighbor;
                    # multi-wide offset APs mis-execute on hardware)
                    gs = []
                    for c in range(3):
                        g = gpool.tile([PT, C], f32, tag=f"g{c}")
                        nc.gpsimd.indirect_dma_start(
                            out=g[:], out_offset=None,
                            in_=v_d[:],
                            in_offset=bass.IndirectOffsetOnAxis(
                                ap=idx8[:, c:c + 1], axis=0,
                            ),
                        )
                        gs.append(g)

                    acc = opool.tile([PT, C], f32, tag="acc")
                    nc.scalar.activation(
                        out=acc[:], in_=gs[0][:],
                        func=mybir.ActivationFunctionType.Copy,
                        scale=w12[:, 3 * t:3 * t + 1],
                    )
                    for c in (1, 2):
                        tt = opool.tile([PT, C], f32, tag=f"t{c}")
                        nc.scalar.activation(
                            out=tt[:], in_=gs[c][:],
                            func=mybir.ActivationFunctionType.Copy,
                            scale=w12[:, 3 * t + c:3 * t + c + 1],
                        )
                        nc.gpsimd.tensor_tensor(
                            out=acc[:], in0=acc[:], in1=tt[:],
                            op=mybir.AluOpType.add,
                        )

                    # 6. transpose [q, c] -> [c, q] and store
                    for h in range(2):
                        tp = psum_tp.tile([PT, PT], f32, tag="tp")
                        nc.tensor.transpose(
                            out=tp[:], in_=acc[:, h * PT:(h + 1) * PT],
                            identity=ident[:],
                        )
                        ot = opool.tile([PT, PT], f32, tag=f"ot{h}")
                        nc.scalar.copy(out=ot[:], in_=tp[:])
                        nc.sync.dma_start(
                            out_d[h * PT:(h + 1) * PT, i * PT:(i + 1) * PT], ot[:],
                        )

    nc.compile()
    return nc


def _split2(x):
    hi = x.astype(np.float16)
    lo = (x - hi.astype(np.float32)).astype(np.float16)
    return hi, lo


def _split3(x):
    hi = x.astype(np.float16)
    mid = (x - hi.astype(np.float32)).astype(np.float16)
    lo = (x - hi.astype(np.float32) - mid.astype(np.float32)).astype(np.float16)
    return hi, mid, lo


def _host_prep(q, k, v):
    """Build per-core input maps (fp16 3-way-split augmented rows).

    m = 2 q.k - |k|^2 with error ~1e-6 (fp32-class): products kept down to
    2^-33 relative: a_hi*(b_hi,b_mid,b_lo), a_mid*(b_hi,b_mid), a_lo*b_hi,
    plus a 3-way split of -|k|^2 against ones. 6*3 + 3 = 21 rows.
    """
    in_maps = []
    for core in range(NCORES):
        b, h = divmod(core, 2)
        qc = np.ascontiguousarray(q[b, h * NSH:(h + 1) * NSH]).astype(np.float32)
        ah, am, al = _split3(qc)
        ones = np.ones((1, NSH), np.float16)

        kb = (2.0 * k[b]).astype(np.float32)
        bh, bm, bl = _split3(kb)
        kk = -np.sum(k[b].astype(np.float32) * k[b].astype(np.float32), axis=-1)
        ch, cm, cl = _split3(kk)

        pairs = [(ah, bh), (ah, bm), (ah, bl), (am, bh), (am, bm), (al, bh)]
        a = np.concatenate(
            [p[0].T for p in pairs] + [ones, ones, ones], axis=0
        )  # [21, NSH]
        kaug = np.concatenate(
            [p[1].T for p in pairs] + [ch[None], cm[None], cl[None]], axis=0
        )  # [21, S]

        qq = np.sum(qc * qc, axis=-1) + 1e-8  # [NSH]
        qq_t = np.ascontiguousarray(qq.reshape(NT, PT).T)  # [128, NT]

        in_maps.append({
            "a": np.ascontiguousarray(a),
            "kaug": np.ascontiguousarray(kaug),
            "qq": qq_t.astype(np.float32),
            "v": np.ascontiguousarray(v[b]).astype(np.float32),
        })
    return in_maps


LAST_RESULTS = None


def _ensure_ntff_hook_importable():
    """bass_utils imports antenv.axon_hooks when tracing is requested; some
    images lack that module. Provide it (wired to libaxon_pjrt if present)."""
    import sys, types
    try:
        import antenv.axon_hooks  # noqa: F401
        return
    except Exception:
        pass
    try:
        import antenv
    except Exception:
        return
    mod = types.ModuleType("antenv.axon_hooks")
    try:
        from trn_agent_boot.trn_boot import _ntff_profile_via_ctypes
        _hook = _ntff_profile_via_ctypes("/opt/axon/libaxon_pjrt.so")
    except Exception:
        _hook = None
    mod.get_axon_ntff_profile_hook = lambda: _hook
    mod.set_axon_ntff_profile_hook = lambda h: None
    sys.modules["antenv.axon_hooks"] = mod
    antenv.axon_hooks = mod


def kernel(q, k, v):
    global LAST_RESULTS
    _ensure_ntff_hook_importable()
    from concourse import bass_utils

    if "nc" not in _CACHE:
        _CACHE["nc"] = _build_bass()
    nc = _CACHE["nc"]

    in_maps = _host_prep(np.asarray(q), np.asarray(k), np.asarray(v))
    res = bass_utils.run_bass_kernel_spmd(
        nc, in_maps, core_ids=list(range(NCORES)),
    )
    LAST_RESULTS = res

    full = np.empty((B, C, N), np.float32)
    for core in range(NCORES):
        b, h = divmod(core, 2)
        full[b, :, h * NSH:(h + 1) * NSH] = res.results[core]["out"]
    return full


# revision 21
# speedup vs baseline: 1.4500x; 1.4500x over previous
"""PointUpsampleAttn (3-NN gather attention) Trainium2 kernel.

Full-input contract: kernel(q, k, v) -> [B, C, N] float32.
  q [4, 16384, 3], k [4, 4096, 3], v [4, 4096, 256]

Sharding: B*N = 65536 queries split across 8 cores (8192 each); core c
handles batch c//2, query half c%2. k/v replicated per-batch (each core
only needs its own batch's k/v). No cross-core reduction.

Per-core kernel, per 128-query tile:
  1. PE matmul (K=11, fp16 hi/lo split of q / 2k / -|k|^2) -> m = 2qk-kk
     in PSUM [128, 4096], fp32-class accuracy at full bf16 PE rate.
  2. ACT copies PSUM -> SBUF.
  3. DVE max8 + max_index -> top-3 m values + s-indices.
  4. weights w = normalize(1/(qq+eps-m_top3)).
  5. 3x indirect DMA gather of v rows; weighted sum; PE transpose to
     [C, n] layout; DMA to output.
"""

import numpy as np

B, N, S, C = 4, 16384, 4096, 256
NCORES = 8
NSH = (B * N) // NCORES   # 8192 queries per core
PT = 128                  # queries per tile (partition dim)
NT = NSH // PT            # 64 tiles
KROWS = 21                # contraction rows of the split matmul

_CACHE = {}


def _build_bass():
    import concourse.bacc as bacc
    import concourse.mybir as mybir
    import concourse.tile as tile
    from concourse import bass
    from concourse.masks import make_identity

    f32 = mybir.dt.float32
    f16 = mybir.dt.float16
    u32 = mybir.dt.uint32

    nc = bacc.Bacc("TRN2", target_bir_lowering=False, debug=False)

    a_d = nc.dram_tensor("a", [KROWS, NSH], f16, kind="ExternalInput").ap()
    k_d = nc.dram_tensor("kaug", [KROWS, S], f16, kind="ExternalInput").ap()
    qq_d = nc.dram_tensor("qq", [PT, NT], f32, kind="ExternalInput").ap()
    v_d = nc.dram_tensor("v", [S, C], f32, kind="ExternalInput").ap()
    out_d = nc.dram_tensor("out", [C, NSH], f32, kind="ExternalOutput").ap()

    with tile.TileContext(nc) as tc:
        with (
            tc.tile_pool(name="const", bufs=1) as cpool,
            tc.tile_pool(name="m", bufs=2) as mpool,
            tc.tile_pool(name="sel", bufs=3) as spool,
            tc.tile_pool(name="g", bufs=3) as gpool,
            tc.tile_pool(name="o", bufs=3) as opool,
            tc.tile_pool(name="mm", bufs=3, space="PSUM") as psum_mm,
            tc.tile_pool(name="tp", bufs=2, space="PSUM") as psum_tp,
        ):
            a_sb = cpool.tile([KROWS, NSH], f16)
            nc.sync.dma_start(a_sb[:], a_d[:])
            k_sb = cpool.tile([KROWS, S], f16)
            nc.sync.dma_start(k_sb[:], k_d[:])
            qq_sb = cpool.tile([PT, NT], f32)
            nc.sync.dma_start(qq_sb[:], qq_d[:])
            ident = cpool.tile([PT, PT], f32)
            make_identity(nc, ident[:])
            eps1 = cpool.tile([PT, 1], f32)
            nc.gpsimd.memset(eps1[:], 1e-9)

            for i in range(NT):
                # 1. distances: m = 2 q.k - |k|^2 for this tile's 128 queries
                m_sb = mpool.tile([PT, S], f32, tag="m")
                lhsT = a_sb[:, i * PT:(i + 1) * PT]
                for j in range(S // 1024):
                    ps = psum_mm.tile([PT, 1024], f32, tag="mm")
                    for jj in range(2):
                        nc.tensor.matmul(
                            ps[:, jj * 512:(jj + 1) * 512], lhsT,
                            k_sb[:, j * 1024 + jj * 512:j * 1024 + (jj + 1) * 512],
                            start=True, stop=True,
                        )
                    # 2. PSUM -> SBUF on the scalar engine
                    nc.scalar.copy(m_sb[:, j * 1024:(j + 1) * 1024], ps[:])

                # 3. top-8 values + indices (we use the first 3)
                top8 = spool.tile([PT, 8], f32, tag="top8")
                nc.vector.max(out=top8[:], in_=m_sb[:])
                idx8 = spool.tile([PT, 8], u32, tag="idx8")
                nc.vector.max_index(out=idx8[:], in_max=top8[:], in_values=m_sb[:])

                # 4. weights: d = relu(qq+eps - m) + tiny floor; w = norm(1/d)
                d3r = spool.tile([PT, 3], f32, tag="d3r")
                nc.scalar.activation(
                    out=d3r[:], in_=top8[:, 0:3],
                    func=mybir.ActivationFunctionType.Relu,
                    scale=-1.0, bias=qq_sb[:, i:i + 1],
                )
                # d3r >= 0, so relu(d3r + eps) == d3r + eps (the floor)
                d3 = spool.tile([PT, 3], f32, tag="d3")
                nc.scalar.activation(
                    out=d3[:], in_=d3r[:],
                    func=mybir.ActivationFunctionType.Relu,
                    bias=eps1[:],
                )
                r3 = spool.tile([PT, 3], f32, tag="r3")
                nc.vector.reciprocal(r3[:], d3[:])
                z = spool.tile([PT, 1], f32, tag="z")
                nc.vector.tensor_reduce(
                    out=z[:], in_=r3[:], axis=mybir.AxisListType.X,
                    op=mybir.AluOpType.add,
                )
                rz = spool.tile([PT, 1], f32, tag="rz")
                nc.vector.reciprocal(rz[:], z[:])
                w3 = spool.tile([PT, 3], f32, tag="w3")
                nc.scalar.activation(
                    out=w3[:], in_=r3[:],
                    func=mybir.ActivationFunctionType.Copy,
                    scale=rz[:],
                )

                # 5. gather v rows (one indirect DMA per neighbor; multi-wide
                # offset APs mis-execute on hardware), then weighted sum
                gs = []
                for c in range(3):
                    g = gpool.tile([PT, C], f32, tag=f"g{c}")
                    nc.gpsimd.indirect_dma_start(
                        out=g[:], out_offset=None,
                        in_=v_d[:],
                        in_offset=bass.IndirectOffsetOnAxis(
                            ap=idx8[:, c:c + 1], axis=0,
                        ),
                    )
                    gs.append(g)

                acc = opool.tile([PT, C], f32, tag="acc")
                nc.scalar.activation(
                    out=acc[:], in_=gs[0][:],
                    func=mybir.ActivationFunctionType.Copy,
                    scale=w3[:, 0:1],
                )
                for c in (1, 2):
                    tt = opool.tile([PT, C], f32, tag=f"t{c}")
                    nc.scalar.activation(
                        out=tt[:], in_=gs[c][:],
                        func=mybir.ActivationFunctionType.Copy,
                        scale=w3[:, c:c + 1],
                    )
                    nc.gpsimd.tensor_tensor(
                        out=acc[:], in0=acc[:], in1=tt[:],
                        op=mybir.AluOpType.add,
                    )

                # 6. transpose [q, c] -> [c, q] and store
                for h in range(2):
                    tp = psum_tp.tile([PT, PT], f32, tag="tp")
                    nc.tensor.transpose(
                        out=tp[:], in_=acc[:, h * PT:(h + 1) * PT],
                        identity=ident[:],
                    )
                    ot = opool.tile([PT, PT], f32, tag=f"ot{h}")
                    nc.scalar.copy(out=ot[:], in_=tp[:])
                    nc.sync.dma_start(
                        out_d[h * PT:(h + 1) * PT, i * PT:(i + 1) * PT], ot[:],
                    )

    nc.compile()
    return nc


def _split2(x):
    hi = x.astype(np.float16)
    lo = (x - hi.astype(np.float32)).astype(np.float16)
    return hi, lo


def _split3(x):
    hi = x.astype(np.float16)
    mid = (x - hi.astype(np.float32)).astype(np.float16)
    lo = (x - hi.astype(np.float32) - mid.astype(np.float32)).astype(np.float16)
    return hi, mid, lo


def _host_prep(q, k, v):
    """Build per-core input maps (fp16 3-way-split augmented rows).

    m = 2 q.k - |k|^2 with error ~1e-6 (fp32-class): products kept down to
    2^-33 relative: a_hi*(b_hi,b_mid,b_lo), a_mid*(b_hi,b_mid), a_lo*b_hi,
    plus a 3-way split of -|k|^2 against ones. 6*3 + 3 = 21 rows.
    """
    in_maps = []
    for core in range(NCORES):
        b, h = divmod(core, 2)
        qc = np.ascontiguousarray(q[b, h * NSH:(h + 1) * NSH]).astype(np.float32)
        ah, am, al = _split3(qc)
        ones = np.ones((1, NSH), np.float16)

        kb = (2.0 * k[b]).astype(np.float32)
        bh, bm, bl = _split3(kb)
        kk = -np.sum(k[b].astype(np.float32) * k[b].astype(np.float32), axis=-1)
        ch, cm, cl = _split3(kk)

        pairs = [(ah, bh), (ah, bm), (ah, bl), (am, bh), (am, bm), (al, bh)]
        a = np.concatenate(
            [p[0].T for p in pairs] + [ones, ones, ones], axis=0
        )  # [21, NSH]
        kaug = np.concatenate(
            [p[1].T for p in pairs] + [ch[None], cm[None], cl[None]], axis=0
        )  # [21, S]

        qq = np.sum(qc * qc, axis=-1) + 1e-8  # [NSH]
        qq_t = np.ascontiguousarray(qq.reshape(NT, PT).T)  # [128, NT]

        in_maps.append({
            "a": np.ascontiguousarray(a),
            "kaug": np.ascontiguousarray(kaug),
            "qq": qq_t.astype(np.float32),
            "v": np.ascontiguousarray(v[b]).astype(np.float32),
        })
    return in_maps


LAST_RESULTS = None


def _ensure_ntff_hook_importable():
    """bass_utils imports antenv.axon_hooks when tracing is requested; some
    images lack that module. Provide it (wired to libaxon_pjrt if present)."""
    import sys, types
    try:
        import antenv.axon_hooks  # noqa: F401
        return
    except Exception:
        pass
    try:
        import antenv
    except Exception:
        return
    mod = types.ModuleType("antenv.axon_hooks")
    try:
        from trn_agent_boot.trn_boot import _ntff_profile_via_ctypes
        _hook = _ntff_profile_via_ctypes("/opt/axon/libaxon_pjrt.so")
    except Exception:
        _hook = None
    mod.get_axon_ntff_profile_hook = lambda: _hook
    mod.set_axon_ntff_profile_hook = lambda h: None
    sys.modules["antenv.axon_hooks"] = mod
    antenv.axon_hooks = mod


def kernel(q, k, v):
    global LAST_RESULTS
    _ensure_ntff_hook_importable()
    from concourse import bass_utils

    if "nc" not in _CACHE:
        _CACHE["nc"] = _build_bass()
    nc = _CACHE["nc"]

    in_maps = _host_prep(np.asarray(q), np.asarray(k), np.asarray(v))
    res = bass_utils.run_bass_kernel_spmd(
        nc, in_maps, core_ids=list(range(NCORES)),
    )
    LAST_RESULTS = res

    full = np.empty((B, C, N), np.float32)
    for core in range(NCORES):
        b, h = divmod(core, 2)
        full[b, :, h * NSH:(h + 1) * NSH] = res.results[core]["out"]
    return full
